# revision 1
# baseline (speedup 1.0000x reference)
"""NT-Xent loss kernel for Trainium2, 8-core SPMD.

Math (matches the reference exactly):
  reps = concat(z_i, z_j)                       [2B, C], B=4096, C=128
  rhat = reps / ||reps||                        (row L2 normalize)
  sim  = rhat @ rhat.T                          [2B, 2B]  (never materialized)
  pos_r = sim[r, (r+B) mod 2B]
  row logits = [pos_r, sim_r(with diag=-inf)] / T,  T=0.5
  loss = mean_r( logsumexp(row) - pos_r/T )
       = mean_r( ln(S_r - exp(2*d_r) + exp(2*pos_r)) - 2*pos_r )
  where S_r = sum_c exp(2 * rhat_r . rhat_c)  (includes diag + pos once)
        d_r = rhat_r . rhat_r  (~1; cancels the masked diagonal term)

Sharding: each of 8 cores owns 1024 query rows (contiguous block k) and
computes per-row (lse_r - 2*pos_r), reduced on-device to a [128,1]
per-partition partial; host sums 8x128 values / 2B. Core k's positive
partner block is block (k+4) % 8.

Perf design (v5; v1 fp32 baseline 202us, v4 124us):
  - matmul operands in float32r (~13-bit mantissa): 1 cyc/col vs 4 for
    fp32 -> 4x faster main loop, 2x faster PE transposes. pos/corr stay
    full fp32 on DVE.
  - query rows stay RAW: the 1/|q_r| factor rides the ScalarE exp's
    per-partition scale AP (out = exp(scale_r * raw_sim)), so qT needs
    no norms -> the q path runs as soon as its DMA lands.
  - exp row-sums via ScalarE accum_out (free reduce along keys).
  - one ACT table set (natural_log_exp_and_others) for Exp+Ln+rsqrt
    (rsqrt = exp(-0.5*ln)): a single ACT_TABLE_LOAD instead of 20.
  - input DMAs alternate the two HWDGE rings (sync + scalar).
  - all squares/reduces on DVE (GpSimd elementwise stalls DVE via SBUF
    port sharing -- measured 1.9us/op on both when concurrent); late
    chunks' norms are emitted mid-main-loop where DVE idles.
  - 16 transposes share one 4-bank PSUM tile -> one wide DVE copy each;
    group G+1 setup is emitted between group G's m-tiles.
"""

import os

import numpy as np

import concourse.bacc as bacc
import concourse.bass as bass
import concourse.mybir as mybir
from concourse.bass_utils import run_bass_kernel_spmd
from concourse.masks import make_identity
from concourse.tile import TileContext

F32 = mybir.dt.float32
F32R = mybir.dt.float32r
AF = mybir.ActivationFunctionType
ALU = mybir.AluOpType
AX = mybir.AxisListType

B = 4096
C = 128
TWOB = 2 * B            # 8192 total rows
N_CORES = 8
M_LOCAL = TWOB // N_CORES   # 1024 query rows per core
MT = M_LOCAL // 128         # 8 m-tiles of 128 queries
KT = TWOB // 128            # 64 key tiles of 128 rows
SPAN = 2048                 # ScalarE exp span = 4 PSUM banks
NG = TWOB // SPAN           # 4 column groups (16 key tiles each)
TPG = SPAN // 128           # 16 key tiles per column group
NCHUNK = 8                  # keys DMA chunks
TPC = KT // NCHUNK          # 8 key tiles per chunk
ISCALE = 2.0                # 1 / temperature

# nrm/inv column layout: 0:8 q | 8:72 keys (chunk g at 8+g*8) | 72:80 p
QC = 0
KC = MT
PC = MT + KT


def _patch_act_tables():
    """Leave Exp/Ln only in natural_log_exp_and_others so bacc's greedy
    set chooser emits ONE table load for the whole kernel (measured: the
    default choice alternated exp<->ln sets, 21 loads, ~27us)."""
    if getattr(bacc, "_ntx_act_patched", False):
        return
    orig = bacc.get_activation_tables

    def patched(arch):
        out = {}
        for name, fns in orig(arch).items():
            if name != "natural_log_exp_and_others":
                fns = fns - {AF.Exp, AF.Ln}
            out[name] = fns
        return out

    bacc.get_activation_tables = patched
    bacc._ntx_act_patched = True


def build_bass() -> bass.Bass:
    _patch_act_tables()
    nc = bacc.Bacc()
    keys = nc.dram_tensor("keys", [TWOB, C], F32, kind="ExternalInput")
    q = nc.dram_tensor("q", [M_LOCAL, C], F32, kind="ExternalInput")
    p = nc.dram_tensor("p", [M_LOCAL, C], F32, kind="ExternalInput")
    out = nc.dram_tensor("out", [128, 1], F32, kind="ExternalOutput")

    with TileContext(nc) as tc:
        with (
            tc.tile_pool(name="big", bufs=1) as big,
            tc.tile_pool(name="small", bufs=1) as small,
            tc.tile_pool(name="scr", bufs=2) as scr,
            tc.tile_pool(name="ps", bufs=2, space="PSUM") as psp,
        ):
            ident = small.tile([128, 128], F32)
            make_identity(nc, ident[:])
            identr = small.tile([128, 128], F32R)
            nc.vector.tensor_copy(identr[:], ident[:])  # round -> f32r

            # ---- input DMAs, alternating the two HWDGE rings
            qt3 = big.tile([128, MT, C], F32)
            pt3 = big.tile([128, MT, C], F32)
            kt3 = big.tile([128, KT, C], F32)
            nc.sync.dma_start(
                out=qt3[:], in_=q[:].rearrange("(t p) c -> p t c", p=128)
            )
            for g in range(NCHUNK):
                eng = nc.scalar if g % 2 else nc.sync
                eng.dma_start(
                    out=kt3[:, g * TPC : (g + 1) * TPC, :],
                    in_=keys[g * (TPC * 128) : (g + 1) * (TPC * 128), :].rearrange(
                        "(t p) c -> p t c", p=128
                    ),
                )
            nc.scalar.dma_start(
                out=pt3[:], in_=p[:].rearrange("(t p) c -> p t c", p=128)
            )

            # ---- q path: raw rows, rounded to f32r, transposed -> qT.
            # No norm dependency: 1/|q| is applied inside the main exp.
            qr3 = big.tile([128, MT, C], F32R)
            nc.vector.tensor_copy(qr3[:], qt3[:])
            qT = big.tile([128, M_LOCAL], F32R)
            tq = psp.tile([128, SPAN], F32R, tag="ps")
            for t in range(MT):
                nc.tensor.transpose(
                    tq[:, t * 128 : (t + 1) * 128], qr3[:, t, :], identr[:]
                )
            nc.vector.tensor_copy(qT[:], tq[:, 0:M_LOCAL])

            nrm = small.tile([128, 16 + KT], F32)
            inv = small.tile([128, 16 + KT], F32)

            def norms(x3, col, n):
                sq = scr.tile([128, n, C], F32, tag="sq")
                nc.vector.tensor_mul(sq[:], x3[:], x3[:])
                nc.vector.reduce_sum(nrm[:, col : col + n], sq[:], axis=AX.X)

            def rsqrt_batch(col, n):
                nc.scalar.activation(nrm[:, col : col + n], nrm[:, col : col + n], AF.Ln)
                nc.scalar.activation(
                    inv[:, col : col + n], nrm[:, col : col + n], AF.Exp, scale=-0.5
                )

            # head-critical: q (for the exp scale) + chunks 0,1 (group 0)
            norms(qt3, QC, MT)
            norms(kt3[:, 0:TPC, :], KC, TPC)
            norms(kt3[:, TPC : 2 * TPC, :], KC + TPC, TPC)
            rsqrt_batch(QC, 24)
            inv2q = small.tile([128, MT], F32)
            nc.vector.tensor_scalar_mul(inv2q[:], inv[:, 0:MT], ISCALE)

            # ---- main: per column group: scale+transpose 16 key tiles,
            # then 8 m-tiles of (4 matmuls + fused exp/rowsum); later
            # groups' norms and the p path are interleaved where DVE idles
            keysT = big.tile([128, TWOB], F32R)
            kn3 = big.tile([128, KT, C], F32R)
            acc = small.tile([128, MT * NG], F32)
            pos = small.tile([128, MT], F32)
            corr = small.tile([128, MT], F32)

            def transpose_group(G):
                for t in range(G * TPG, (G + 1) * TPG):
                    nc.vector.tensor_scalar_mul(
                        kn3[:, t, :], kt3[:, t, :], inv[:, KC + t : KC + t + 1]
                    )
                tp = psp.tile([128, SPAN], F32R, tag="ps")
                for i, t in enumerate(range(G * TPG, (G + 1) * TPG)):
                    nc.tensor.transpose(
                        tp[:, i * 128 : (i + 1) * 128], kn3[:, t, :], identr[:]
                    )
                nc.vector.tensor_copy(
                    keysT[:, G * SPAN : (G + 1) * SPAN], tp[:]
                )

            transpose_group(0)
            for G in range(NG):
                for m in range(MT):
                    psm = psp.tile([128, SPAN], F32, tag="ps")
                    for j in range(SPAN // 512):
                        col = G * SPAN + j * 512
                        nc.tensor.matmul(
                            psm[:, j * 512 : (j + 1) * 512],
                            lhsT=qT[:, m * 128 : (m + 1) * 128],
                            rhs=keysT[:, col : col + 512],
                            start=True,
                            stop=True,
                        )
                    nc.scalar.activation(
                        psm[:],
                        psm[:],
                        AF.Exp,
                        scale=inv2q[:, m : m + 1],
                        accum_out=acc[:, m * NG + G : m * NG + G + 1],
                    )
                    if m == 1 and G + 1 < NG:
                        g0 = 2 * (G + 1)
                        norms(kt3[:, g0 * TPC : (g0 + 2) * TPC, :], KC + g0 * TPC,
                              2 * TPC)
                        rsqrt_batch(KC + g0 * TPC, 2 * TPC)
                    if m == 3 and G + 1 < NG:
                        transpose_group(G + 1)
                    if G == 1 and m == 5:
                        # p path: only needed for the final pos term
                        norms(pt3, PC, MT)
                        rsqrt_batch(PC, MT)
                    if G == 2 and m == 5:
                        # pos/corr from RAW dots times the inverse norms
                        prod = scr.tile([128, MT, C], F32, tag="sq")
                        nc.vector.tensor_mul(prod[:], qt3[:], pt3[:])
                        nc.vector.reduce_sum(pos[:], prod[:], axis=AX.X)
                        prod2 = scr.tile([128, MT, C], F32, tag="sq")
                        nc.vector.tensor_mul(prod2[:], qt3[:], qt3[:])
                        nc.vector.reduce_sum(corr[:], prod2[:], axis=AX.X)
                        nc.vector.tensor_mul(pos[:], pos[:], inv[:, QC : QC + MT])
                        nc.vector.tensor_mul(pos[:], pos[:], inv[:, PC : PC + MT])
                        nc.vector.tensor_mul(corr[:], corr[:], inv[:, QC : QC + MT])
                        nc.vector.tensor_mul(corr[:], corr[:], inv[:, QC : QC + MT])

            # ---- finalize: loss_r = ln(S - e^{2 corr} + e^{2 pos}) - 2 pos
            S = small.tile([128, MT], F32)
            nc.vector.reduce_sum(
                S[:], acc[:].rearrange("p (m g) -> p m g", g=NG), axis=AX.X
            )
            ecorr = small.tile([128, MT], F32)
            epos = small.tile([128, MT], F32)
            nc.scalar.activation(ecorr[:], corr[:], AF.Exp, scale=ISCALE)
            nc.scalar.activation(epos[:], pos[:], AF.Exp, scale=ISCALE)
            tot = small.tile([128, MT], F32)
            nc.vector.tensor_sub(tot[:], S[:], ecorr[:])
            nc.vector.tensor_add(tot[:], tot[:], epos[:])
            nc.scalar.activation(tot[:], tot[:], AF.Ln)
            rowloss = small.tile([128, MT], F32)
            nc.vector.scalar_tensor_tensor(
                out=rowloss[:],
                in0=pos[:],
                scalar=-ISCALE,
                in1=tot[:],
                op0=ALU.mult,
                op1=ALU.add,
            )
            rsum = small.tile([128, 1], F32)
            nc.vector.reduce_sum(rsum[:], rowloss[:], axis=AX.X)
            nc.sync.dma_start(out=out[:], in_=rsum[:])

    nc.finalize()
    return nc


_NC_CACHE: bass.Bass | None = None
LAST_RESULTS = None  # BassKernelResults of the last run (for profiling)


def _get_nc() -> bass.Bass:
    global _NC_CACHE
    if _NC_CACHE is None:
        _NC_CACHE = build_bass()
    return _NC_CACHE


def kernel(z_i: np.ndarray, z_j: np.ndarray) -> np.ndarray:
    global LAST_RESULTS
    z_i = np.ascontiguousarray(np.asarray(z_i, dtype=np.float32))
    z_j = np.ascontiguousarray(np.asarray(z_j, dtype=np.float32))
    assert z_i.shape == (B, C) and z_j.shape == (B, C)

    reps = np.concatenate([z_i, z_j], axis=0)  # [2B, C]
    in_maps = []
    for k in range(N_CORES):
        kq = reps[k * M_LOCAL : (k + 1) * M_LOCAL]
        kp_blk = (k + N_CORES // 2) % N_CORES
        kp = reps[kp_blk * M_LOCAL : (kp_blk + 1) * M_LOCAL]
        in_maps.append(
            {
                "keys": reps,
                "q": np.ascontiguousarray(kq),
                "p": np.ascontiguousarray(kp),
            }
        )

    nc = _get_nc()
    trace = bool(int(os.environ.get("KERNEL_TRACE", "0")))
    res = run_bass_kernel_spmd(
        nc, in_maps, core_ids=list(range(N_CORES)), trace=trace
    )
    LAST_RESULTS = res
    total = sum(float(r["out"].sum()) for r in res.results)
    return np.float32(total / TWOB)



# revision 12
# speedup vs baseline: 1.7556x; 1.7556x over previous
"""NT-Xent loss kernel for Trainium2, 8-core SPMD — v6 (symmetric triangle).

Math (matches the reference):
  reps = concat(z_i, z_j)  [2B, C], B=4096, C=128; rhat = reps/|reps|
  S_r = sum_c exp(2 rhat_r . rhat_c);  pos_r = rhat_r . rhat_{(r+B)%2B}
  loss = mean_r( ln(S_r - e^2 + e^{2 pos_r}) - 2 pos_r )

exp(2 sim) is symmetric, so each unordered pair is exp'd ONCE:
  Rotation decomposition of the 64x64 block grid (blocks of 128 rows):
  every core runs the SAME canonical program computing tiles (a, a+d),
  a in 0..7, d in 0..31, on keys ROTATED by 8k blocks (host np.roll).
  Across 8 cores this covers every pair with |d| in 0..31 exactly once.
  The d=32 pairs (also the positive pairs) don't decompose under the
  rotation group; they are fed as separate per-core gathered inputs
  q32/p32 (4 tiles per core), whose diagonals give pos_r for free.

Per tile both sums are extracted: row sums ride ScalarE's accum_out;
column sums are accumulated elementwise into a bf16 strip A (DVE adds)
and partition-folded ONCE at the end on the PE (per-tile lhsT=A-tile
matmul against a ones vector). Host scatter-adds the per-core partials
(static maps), subtracts e^2 for the diagonal, and takes ln/mean in
fp64 over the 8192 rows.

Per core: 260 exp tiles (vs 512 full-matrix), 2.44MB keys DMA (only
blocks 0..38 are referenced canonically), bf16 matmuls (1024-wide
moving operand), normalization folded into the transposed copy via a
partition-broadcast row of 1/|k| (one DVE pass).
"""

import os

import numpy as np

import concourse.bacc as bacc
import concourse.bass as bass
import concourse.mybir as mybir
from concourse.bass_utils import run_bass_kernel_spmd
from concourse.masks import make_identity
from concourse.tile import TileContext

F32 = mybir.dt.float32
F32R = mybir.dt.float32r
BF16 = mybir.dt.bfloat16
AF = mybir.ActivationFunctionType
ALU = mybir.AluOpType
AX = mybir.AxisListType

B = 4096
C = 128
TWOB = 2 * B
N_CORES = 8
NB = 64                 # 128-row blocks in the full matrix
NBK = 39                # canonical key blocks each core loads (0..38)
KROWS = NBK * 128       # 4992
ISCALE = 2.0            # 1 / temperature
NA = 8                  # canonical stationary blocks per core (a = 0..7)
SPAN = 2048             # 16 key tiles per exp span; 2 spans per a
CHUNKS = [(0, 8), (8, 8), (16, 8), (24, 8), (32, 7)]  # keys DMA chunks


def _patch_act_tables():
    """Keep Exp/Ln only in natural_log_exp_and_others so bacc emits ONE
    ACT_TABLE_LOAD for the whole kernel."""
    if getattr(bacc, "_ntx_act_patched", False):
        return
    orig = bacc.get_activation_tables

    def patched(arch):
        out = {}
        for name, fns in orig(arch).items():
            if name != "natural_log_exp_and_others":
                fns = fns - {AF.Exp, AF.Ln}
            out[name] = fns
        return out

    bacc.get_activation_tables = patched
    bacc._ntx_act_patched = True


def build_bass() -> bass.Bass:
    _patch_act_tables()
    nc = bacc.Bacc()
    keys = nc.dram_tensor("keys", [KROWS, C], F32, kind="ExternalInput")
    q32 = nc.dram_tensor("q32", [512, C], F32, kind="ExternalInput")
    p32 = nc.dram_tensor("p32", [512, C], F32, kind="ExternalInput")
    cs_o = nc.dram_tensor("cs_o", [128, NBK - 1], F32, kind="ExternalOutput")
    rs_o = nc.dram_tensor("rs_o", [128, 2 * NA], F32, kind="ExternalOutput")
    d32_o = nc.dram_tensor("d32_o", [128, 12], F32, kind="ExternalOutput")

    with TileContext(nc) as tc:
        with (
            tc.tile_pool(name="big", bufs=1) as big,
            tc.tile_pool(name="small", bufs=1) as small,
            tc.tile_pool(name="scr", bufs=2) as scr,
            tc.tile_pool(name="expp", bufs=4) as expp,
            tc.tile_pool(name="ps", bufs=2, space="PSUM") as psp,
        ):
            # ---- input DMAs on otherwise-idle engines (never ScalarE)
            kt3 = big.tile([128, NBK, C], F32)
            dma_engs = [nc.sync, nc.gpsimd, nc.sync, nc.gpsimd, nc.sync]
            for g, (b0, nt) in enumerate(CHUNKS):
                dma_engs[g].dma_start(
                    out=kt3[:, b0 : b0 + nt, :],
                    in_=keys[b0 * 128 : (b0 + nt) * 128, :].rearrange(
                        "(t p) c -> p t c", p=128
                    ),
                )
            q32t = small.tile([128, 4, C], F32)
            p32t = small.tile([128, 4, C], F32)
            nc.gpsimd.dma_start(
                out=q32t[:], in_=q32[:].rearrange("(t p) c -> p t c", p=128)
            )
            nc.sync.dma_start(
                out=p32t[:], in_=p32[:].rearrange("(t p) c -> p t c", p=128)
            )

            ident = small.tile([128, 128], F32)
            make_identity(nc, ident[:])
            identb = small.tile([128, 128], BF16)
            nc.vector.tensor_copy(identb[:], ident[:])
            onesb = small.tile([128, 1], BF16)
            nc.vector.memset(onesb[:], 1.0)

            # cs accumulator strip: canonical key cols 1..38
            A = big.tile([128, (NBK - 1) * 128], BF16)
            nc.gpsimd.memset(A[:], 0.0)

            # ---- norms: nrm[:, 0:39] keys, 39:43 q32, 43:47 p32
            nrm = small.tile([128, 48], F32)
            inv = small.tile([128, 48], F32)

            def norms(x3, col, n):
                sq = scr.tile([128, 16, C], F32, tag="sq")
                nc.vector.tensor_mul(sq[:, 0:n, :], x3[:], x3[:])
                nc.vector.reduce_sum(nrm[:, col : col + n], sq[:, 0:n, :], axis=AX.X)

            def rsqrt_batch(col, n):
                nc.scalar.activation(nrm[:, col : col + n], nrm[:, col : col + n], AF.Ln)
                nc.scalar.activation(
                    inv[:, col : col + n], nrm[:, col : col + n], AF.Exp, scale=-0.5
                )

            keysT = big.tile([128, KROWS], BF16)
            kn3 = big.tile([128, NBK, C], BF16)

            def transpose_scale(g):
                b0, nt = CHUNKS[g]
                nc.vector.tensor_mul(
                    kn3[:, b0 : b0 + nt, :],
                    kt3[:, b0 : b0 + nt, :],
                    inv[:, b0 : b0 + nt].unsqueeze(2).broadcast_to((128, nt, C)),
                )
                tq = psp.tile([128, 1024], BF16, tag="ps")
                for i in range(nt):
                    nc.tensor.transpose(
                        tq[:, i * 128 : (i + 1) * 128], kn3[:, b0 + i, :], identb[:]
                    )
                nc.vector.tensor_copy(
                    keysT[:, b0 * 128 : (b0 + nt) * 128], tq[:, 0 : nt * 128]
                )

            # head-critical: chunks 0+1 first
            norms(kt3[:, 0:16, :], 0, 16)
            rsqrt_batch(0, 16)
            transpose_scale(0)
            transpose_scale(1)

            rs = small.tile([128, 2 * NA], F32)

            def span(a, s):
                c0 = (a + 16 * s) * 128
                psm = psp.tile([128, SPAN], F32, tag="ps")
                lhs = keysT[:, a * 128 : (a + 1) * 128]
                for j in range(SPAN // 512):
                    nc.tensor.matmul(
                        psm[:, j * 512 : (j + 1) * 512],
                        lhsT=lhs,
                        rhs=keysT[:, c0 + j * 512 : c0 + (j + 1) * 512],
                        start=True, stop=True,
                    )
                expb = expp.tile([128, SPAN], BF16, tag="eb")
                nc.scalar.activation(
                    expb[:], psm[:], AF.Exp, scale=ISCALE,
                    accum_out=rs[:, 2 * a + s : 2 * a + s + 1],
                )
                if s == 0:  # cols a+1..a+15 (skip diag tile a)
                    nc.vector.tensor_add(
                        A[:, a * 128 : (a + 15) * 128],
                        A[:, a * 128 : (a + 15) * 128],
                        expb[:, 128:2048],
                    )
                else:  # cols a+16..a+31
                    nc.vector.tensor_add(
                        A[:, (a + 15) * 128 : (a + 31) * 128],
                        A[:, (a + 15) * 128 : (a + 31) * 128],
                        expb[:],
                    )

            d32out = small.tile([128, 12], F32)
            exp32 = small.tile([128, 512], BF16)

            def d32_prep():
                norms(q32t, 39, 4)
                norms(p32t, 43, 4)
                rsqrt_batch(39, 8)
                q32n = scr.tile([128, 4, C], BF16, tag="d32n")
                p32n = scr.tile([128, 4, C], BF16, tag="d32n")
                nc.vector.tensor_mul(
                    q32n[:], q32t[:],
                    inv[:, 39:43].unsqueeze(2).broadcast_to((128, 4, C)),
                )
                nc.vector.tensor_mul(
                    p32n[:], p32t[:],
                    inv[:, 43:47].unsqueeze(2).broadcast_to((128, 4, C)),
                )
                tp = psp.tile([128, 1024], BF16, tag="ps")
                for t in range(4):
                    nc.tensor.transpose(
                        tp[:, t * 128 : (t + 1) * 128], q32n[:, t, :], identb[:]
                    )
                for t in range(4):
                    nc.tensor.transpose(
                        tp[:, 512 + t * 128 : 512 + (t + 1) * 128], p32n[:, t, :],
                        identb[:],
                    )
                qpT = small.tile([128, 1024], BF16)
                nc.vector.tensor_copy(qpT[:], tp[:])
                return qpT

            def d32_main(qpT):
                psm32 = psp.tile([128, 512], F32, tag="ps")
                for t in range(4):
                    nc.tensor.matmul(
                        psm32[:, t * 128 : (t + 1) * 128],
                        lhsT=qpT[:, t * 128 : (t + 1) * 128],
                        rhs=qpT[:, 512 + t * 128 : 512 + (t + 1) * 128],
                        start=True, stop=True,
                    )
                # pos32 = diagonals of the 4 normalized sim tiles
                dsc = scr.tile([128, 4, 128], F32, tag="dsc")
                nc.vector.tensor_mul(
                    dsc[:],
                    psm32[:].rearrange("p (t f) -> p t f", t=4),
                    ident[:].unsqueeze(1).broadcast_to((128, 4, 128)),
                )
                nc.vector.reduce_sum(d32out[:, 8:12], dsc[:], axis=AX.X)
                nc.scalar.activation(exp32[:], psm32[:], AF.Exp, scale=ISCALE)
                # rs32: free-axis sums on DVE
                nc.vector.reduce_sum(
                    d32out[:, 0:4],
                    exp32[:].rearrange("p (t f) -> p t f", t=4),
                    axis=AX.X,
                )
                # cs32: partition sums via PE fold
                csp32 = psp.tile([128, 4], F32, tag="ps")
                for t in range(4):
                    nc.tensor.matmul(
                        csp32[:, t : t + 1],
                        lhsT=exp32[:, t * 128 : (t + 1) * 128],
                        rhs=onesb[:],
                        start=True, stop=True,
                    )
                nc.vector.tensor_copy(d32out[:, 4:8], csp32[:])

            # ---- main: spans gated on the transpose_scale that feeds them
            # s0 of a needs key blocks <= a+15; s1 needs <= a+31
            span(0, 0)
            norms(kt3[:, 16:32, :], 16, 16)
            rsqrt_batch(16, 16)
            transpose_scale(2)
            span(1, 0)
            transpose_scale(3)
            span(2, 0)
            norms(kt3[:, 32:39, :], 32, 7)
            span(3, 0)
            rsqrt_batch(32, 7)
            span(4, 0)
            transpose_scale(4)
            span(0, 1)
            span(5, 0)
            span(1, 1)
            qpT = d32_prep()
            span(6, 0)
            span(2, 1)
            span(7, 0)
            d32_main(qpT)
            span(3, 1)
            span(4, 1)
            span(5, 1)
            span(6, 1)
            span(7, 1)

            # ---- tail: fold A into per-block column sums on the PE
            csp = psp.tile([128, NBK - 1], F32, tag="ps")
            for c in range(NBK - 1):
                nc.tensor.matmul(
                    csp[:, c : c + 1],
                    lhsT=A[:, c * 128 : (c + 1) * 128],
                    rhs=onesb[:],
                    start=True, stop=True,
                )
            cs_sb = small.tile([128, NBK - 1], F32)
            nc.vector.tensor_copy(cs_sb[:], csp[:])
            nc.sync.dma_start(out=cs_o[:], in_=cs_sb[:])
            nc.gpsimd.dma_start(out=rs_o[:], in_=rs[:])
            nc.sync.dma_start(out=d32_o[:], in_=d32out[:])

    nc.finalize()
    return nc


_NC_CACHE: bass.Bass | None = None
LAST_RESULTS = None  # BassKernelResults of the last run (for profiling)


def _get_nc() -> bass.Bass:
    global _NC_CACHE
    if _NC_CACHE is None:
        _NC_CACHE = build_bass()
    return _NC_CACHE


def kernel(z_i: np.ndarray, z_j: np.ndarray) -> np.ndarray:
    global LAST_RESULTS
    z_i = np.asarray(z_i, dtype=np.float32)
    z_j = np.asarray(z_j, dtype=np.float32)
    assert z_i.shape == (B, C) and z_j.shape == (B, C)

    reps = np.concatenate([z_i, z_j], axis=0)  # [2B, C]
    reps64 = reps.reshape(NB, 128, C)
    in_maps = []
    for k in range(N_CORES):
        rot = np.roll(reps64, -8 * k, axis=0)
        in_maps.append(
            {
                "keys": np.ascontiguousarray(rot[:NBK].reshape(KROWS, C)),
                "q32": np.ascontiguousarray(
                    reps64[4 * k : 4 * k + 4].reshape(512, C)
                ),
                "p32": np.ascontiguousarray(
                    reps64[4 * k + 32 : 4 * k + 36].reshape(512, C)
                ),
            }
        )

    nc = _get_nc()
    trace = bool(int(os.environ.get("KERNEL_TRACE", "0")))
    res = run_bass_kernel_spmd(
        nc, in_maps, core_ids=list(range(N_CORES)), trace=trace
    )
    LAST_RESULTS = res

    # ---- host assembly (tiny: 8192-row scatter + ln in fp64)
    S64 = np.zeros((NB, 128), dtype=np.float64)
    pos = np.zeros((NB, 128), dtype=np.float64)
    for k in range(N_CORES):
        r = res.results[k]
        cs = np.asarray(r["cs_o"], dtype=np.float64)      # [128, 38]
        rs = np.asarray(r["rs_o"], dtype=np.float64)      # [128, 16]
        d32 = np.asarray(r["d32_o"], dtype=np.float64)    # [128, 12]
        ccols = (np.arange(1, NBK) + 8 * k) % NB          # 38 global blocks
        np.add.at(S64, ccols, cs.T)
        acols = (np.arange(NA) + 8 * k) % NB              # 8 global blocks
        np.add.at(S64, acols, (rs[:, 0::2] + rs[:, 1::2]).T)
        a32 = 4 * k + np.arange(4)
        b32 = a32 + 32
        np.add.at(S64, a32, d32[:, 0:4].T)                # rs32
        np.add.at(S64, b32, d32[:, 4:8].T)                # cs32
        pos[a32] = d32[:, 8:12].T
        pos[b32] = d32[:, 8:12].T

    Sv = S64.reshape(TWOB)
    pv = pos.reshape(TWOB)
    tot = Sv - np.exp(2.0) + np.exp(2.0 * pv)
    loss = np.mean(np.log(tot) - 2.0 * pv)
    return np.float32(loss)
